# revision 20
# baseline (speedup 1.0000x reference)
"""Trainium2 Bass kernel for nn_ClassBalance (argmax-histogram + EMA epilogue).

Input : generated_masks [16, 8, 512, 512] f32, prev_dist [8] f32
Output: (balance scalar f32, class_distribution [8] f32)

Strategy (8 NeuronCores, data-parallel over batch):
  - core k processes batches [2k, 2k+1] (contiguous 16 MiB shard)
  - per batch image [8, 512*512]: DMA each channel plane into a
    [128, 2048] f32 SBUF tile; ScalarE/GpSimd convert f32->bf16; DVE
    computes the 8-way channel max (tensor_tensor max tree, bf16 2x
    mode) and per-channel is_ge(x_c, max) winner masks (channels 0-6;
    channel 7 is derived on the host from the pixel total); the PE
    reduces each mask against a ones-vector into PSUM bank c,
    accumulating over column blocks and both batches; per-channel DVE
    reduces then yield the per-core histogram.
  - host sums the 8 per-core histograms (all-reduce) and runs the
    O(num_classes) EMA + norm epilogue in f32.
"""

import numpy as np
from contextlib import ExitStack

import concourse.bass as bass
import concourse.tile as tile
from concourse import bacc, mybir
from concourse.bass_utils import run_bass_kernel_spmd

B, C, H, W = 16, 8, 512, 512
N_CORES = 8
BPC = B // N_CORES            # batches per core
PLANE = H * W                 # 262144 pixels per channel plane
P = 128                       # SBUF partitions
FREE = PLANE // P             # 2048 free-dim elements per tile
MM_N = 512                    # matmul moving free-dim limit
NBLK = FREE // MM_N
CC = C - 1                    # channels counted on-device (last via total)
EMA_W = 0.99

F32 = mybir.dt.float32
BF16 = mybir.dt.bfloat16

_NC = None          # cached Bass program (compile once per process)
LAST_RESULTS = None  # BassKernelResults of the most recent run (for profiling)
TRACE = False        # set True before calling kernel() to capture an NTFF profile


def _build_nc():
    # Bacc (not raw Bass): its compile() splits multi-wait instructions via
    # event semaphores — TRN2 allows at most one sync wait per instruction.
    nc = bacc.Bacc(
        "TRN2", target_bir_lowering=False, debug=False, num_devices=N_CORES
    )
    x = nc.dram_tensor("x", [BPC * C, PLANE], F32, kind="ExternalInput")
    hist = nc.dram_tensor("hist", [1, CC], F32, kind="ExternalOutput")

    # column slices per plane: small first slice so compute starts early,
    # small last slice so the last-DMA-gated serial chain is short
    SLICES = [(0, 512), (512, 512), (1024, 512), (1536, 512)]
    # the final chunk tapers so the last-DMA-gated serial chain is shorter
    SLICES_LAST = [(0, 512), (512, 512), (1024, 512), (1536, 512)]

    with ExitStack() as ctx:
        tc = ctx.enter_context(tile.TileContext(nc))
        xf_pool = ctx.enter_context(tc.tile_pool(name="xf", bufs=10))
        xb_pool = ctx.enter_context(tc.tile_pool(name="xb", bufs=20))
        ge_pool = ctx.enter_context(tc.tile_pool(name="ge", bufs=6))
        mx_pool = ctx.enter_context(tc.tile_pool(name="mx", bufs=12))
        sm_pool = ctx.enter_context(tc.tile_pool(name="sm", bufs=1))
        psum_pool = ctx.enter_context(
            tc.tile_pool(name="psum", bufs=1, space=bass.MemorySpace.PSUM)
        )

        ones = sm_pool.tile([P, 1], BF16, tag="ones")
        nc.gpsimd.memset(ones[:], 1.0)
        cnt = sm_pool.tile([1, CC], F32, tag="cnt")
        red_scratch = sm_pool.tile([1, CC, MM_N], F32, tag="redscr")
        # channel c accumulates into PSUM bank c (region [1, MM_N] f32 = 2 KiB)
        ps = psum_pool.tile([1, CC, MM_N], F32)

        slice_plan = [SLICES] * (BPC - 1) + [SLICES_LAST]
        n_groups = sum(len(s) for s in slice_plan)
        gi = -1
        for b in range(BPC):
            for si, (off, sz) in enumerate(slice_plan[b]):
                gi += 1
                planes = [x[b * C + c].rearrange("(p f) -> p f", p=P) for c in range(C)]
                xb = []
                for c in range(C):
                    xf = xf_pool.tile([P, sz], F32, tag="xf")
                    nc.sync.dma_start(xf[:], planes[c][:, off : off + sz])
                    t = xb_pool.tile([P, sz], BF16, tag="xb")
                    # split f32->bf16 conversions between ScalarE and GpSimd
                    if c in (2, 5, 7):
                        nc.gpsimd.tensor_copy(t[:], xf[:])
                    else:
                        nc.scalar.copy(t[:], xf[:])
                    xb.append(t)

                # 8-way max tree on DVE (bf16 2x mode)
                lvl = xb
                while len(lvl) > 1:
                    nxt = []
                    for i in range(0, len(lvl), 2):
                        m = mx_pool.tile([P, sz], BF16, tag="mx")
                        nc.vector.tensor_tensor(
                            m[:], lvl[i][:], lvl[i + 1][:], mybir.AluOpType.max
                        )
                        nxt.append(m)
                    lvl = nxt
                mfull = lvl[0]

                # per-channel winner masks + PE reduction into PSUM
                for c in range(CC):
                    ge = ge_pool.tile([P, sz], BF16, tag="ge")
                    nc.vector.tensor_tensor(
                        ge[:], xb[c][:], mfull[:], mybir.AluOpType.is_ge
                    )
                    for j0 in range(0, sz, MM_N):
                        w = min(MM_N, sz - j0)
                        # partial-width blocks accumulate into columns [0, w)
                        # of bank c; only the column-wise sum matters
                        nc.tensor.matmul(
                            ps[:, c, 0:w],
                            ones[:],
                            ge[:, j0 : j0 + w],
                            start=(gi == 0 and j0 == 0),
                            stop=(gi == n_groups - 1 and j0 + w == sz),
                            skip_group_check=True,
                        )
                    if gi == n_groups - 1:
                        # per-channel final reduce right after its last matmul
                        # (on ScalarE via activation-accumulate: it is idle at
                        # the tail and is the PSUM-near engine)
                        if c % 2 == 0:
                            nc.scalar.activation(
                                red_scratch[:, c, :],
                                ps[:, c, :],
                                mybir.ActivationFunctionType.Copy,
                                accum_out=cnt[:, c : c + 1],
                            )
                        else:
                            nc.vector.tensor_reduce(
                                cnt[:, c : c + 1],
                                ps[:, c, :],
                                axis=mybir.AxisListType.X,
                                op=mybir.AluOpType.add,
                            )

        nc.sync.dma_start(hist[:], cnt[:])

    nc.compile()
    return nc


def kernel(generated_masks, prev_dist):
    global _NC, LAST_RESULTS
    gm = np.ascontiguousarray(np.asarray(generated_masks, dtype=np.float32))
    pd = np.asarray(prev_dist, dtype=np.float32)
    assert gm.shape == (B, C, H, W)

    if _NC is None:
        _NC = _build_nc()

    planes = gm.reshape(B * C, PLANE)
    in_maps = [
        {"x": planes[k * BPC * C : (k + 1) * BPC * C]} for k in range(N_CORES)
    ]
    LAST_RESULTS = run_bass_kernel_spmd(
        _NC, in_maps, core_ids=list(range(N_CORES)), trace=TRACE
    )
    per_core_pixels = np.float32(BPC * PLANE)
    hists = []
    for k in range(N_CORES):
        h7 = LAST_RESULTS.results[k]["hist"].reshape(CC)
        hists.append(np.concatenate([h7, [per_core_pixels - h7.sum(dtype=np.float32)]]))

    # all-reduce across cores + EMA/norm epilogue (O(num_classes), host-side)
    hist_full = np.sum(np.stack(hists), axis=0, dtype=np.float32)
    total = np.float32(B * H * W)
    norm_factor = np.float32(1.0 / C)
    class_distribution = (
        pd * np.float32(EMA_W) + np.float32(1.0 - EMA_W) * hist_full / total
    ).astype(np.float32)
    balance = np.linalg.norm(
        (class_distribution - norm_factor) / (np.float32(1.0) - norm_factor)
    ).astype(np.float32)
    return balance, class_distribution
